# revision 15
# baseline (speedup 1.0000x reference)
"""Trainium2 Bass kernel for pointer-generator final-distribution (scatter_memory).

out[r, v] = p_gens[r] * vocab_ds[r, v]  (+ (1-p_gens[r])*attns[r, l_win]  at
v == sources[l, b(r)], duplicate source ids resolved last-occurrence-wins)

Strategy (8 NeuronCores, SPMD):
  - Shard by batch column: core k owns b in {4k..4k+3}, all T decoder steps
    (rows r = t*B + b). Host pre-gathers rows b-major so device DMAs are
    contiguous; two 128-row groups per core (2 b's x 64 t each).
  - Stream out = p * vocab through SBUF ([128, 4096] tiles, ACT does the
    per-partition scale).
  - The scatter is applied in SBUF before the store via a compact one-hot
    matmul on the (otherwise idle) PE: for each 512-wide subtile, host bakes
    a [K, 128] block of update values (update k x row, block-diagonal over
    the two b's) and relative target columns ck; device builds the one-hot
    [K, 512] with is_equal(iota, ck) and computes proj = vals.T @ onehot in
    PSUM, then DVE adds it into the streamed tile. All products are 1.0*val
    or 0.0*val so the result is exact f32.
"""

import numpy as np

N_CORES = 8
WIN = 4096
SUB = 512


def _host_prep(vocab_ds, attns, p_gens, sources, T):
    f32 = np.float32
    vocab_ds = np.ascontiguousarray(vocab_ds, dtype=f32)
    attns = np.ascontiguousarray(attns, dtype=f32)
    p_gens = np.ascontiguousarray(p_gens, dtype=f32)
    src = np.asarray(sources).astype(np.int64)
    rows, V = vocab_ds.shape
    L, B = src.shape
    assert rows == T * B

    ag = (f32(1.0) - p_gens) * attns  # gated copy dist, [rows, L]

    # winners per batch column: duplicate source ids -> last occurrence wins
    wins = []
    for b in range(B):
        d = {}
        col = src[:, b]
        for l in range(L):
            d[int(col[l])] = l
        cols = np.fromiter(d.keys(), dtype=np.int64)
        ls = np.fromiter(d.values(), dtype=np.int64)
        o = np.argsort(cols)
        wins.append((cols[o], ls[o]))

    NW = (V + WIN - 1) // WIN
    # subtile geometry, shared by all cores/groups
    sub_geom = []  # (w, s, c0_abs, width)
    for w in range(NW):
        ww = min(WIN, V - w * WIN)
        for s in range((ww + SUB - 1) // SUB):
            sub_geom.append((w, s, w * WIN + s * SUB, min(SUB, ww - s * SUB)))
    NS = len(sub_geom)
    sub_of = {}
    for i, (w, s, c0, wd) in enumerate(sub_geom):
        sub_of[(w, s)] = i

    BPC = B // N_CORES  # 4
    G = BPC // 2        # 2 groups of 2 b's

    # bucket updates per (core, g, subtile)
    upd = [[[[] for _ in range(NS)] for _ in range(G)] for _ in range(N_CORES)]
    for core in range(N_CORES):
        for g in range(G):
            for half in range(2):
                b = core * BPC + g * 2 + half
                cols, ls = wins[b]
                for c, l in zip(cols.tolist(), ls.tolist()):
                    w = c // WIN
                    s = (c - w * WIN) // SUB
                    i = sub_of[(w, s)]
                    upd[core][g][i].append((half, c, l))

    # uniform-per-(g, subtile) K across cores
    K_ws = [[max(len(upd[core][g][i]) for core in range(N_CORES)) for i in range(NS)]
            for g in range(G)]
    KMAX = [max(K_ws[g]) if NS else 0 for g in range(G)]
    # per-(g, window): first subtile index, #subtiles, max 3K (partition
    # extent of the JIT vals load for that window; 3x for hi/mid/lo split)
    win_info = []
    for g in range(G):
        wi = []
        for w in range(NW):
            idxs = [i for i, (w2, s2, _, _) in enumerate(sub_geom) if w2 == w]
            i0, nsub = idxs[0], len(idxs)
            kw = max(3 * K_ws[g][i] for i in idxs)
            wi.append((i0, nsub, kw))
        win_info.append(wi)

    # per-core device inputs
    in_maps = []
    for core in range(N_CORES):
        m = {}
        for g in range(G):
            row_idx = []
            for half in range(2):
                b = core * BPC + g * 2 + half
                row_idx.extend(t * B + b for t in range(T))
            row_idx = np.asarray(row_idx)
            m[f"vocab{g}"] = vocab_ds[row_idx]
            m[f"pgen{g}"] = p_gens[row_idx]
            import ml_dtypes
            bf16 = ml_dtypes.bfloat16
            vals = np.zeros((128, NS * 128), dtype=f32)
            ck = np.full((128, NS), -1.0, dtype=f32)
            for i in range(NS):
                w, s, c0, wd = sub_geom[i]
                for k, (half, c, l) in enumerate(upd[core][g][i]):
                    # rows of this b occupy partitions half*T .. half*T+T
                    r0 = half * T
                    vals[k, i * 128 + r0: i * 128 + r0 + T] = ag[row_idx[r0: r0 + T], l]
                    kw = K_ws[g][i]
                    ck[k, i] = f32(c - c0)
                    ck[kw + k, i] = f32(c - c0)
                    ck[2 * kw + k, i] = f32(c - c0)
            # exact 3-way bf16 split: val = hi + mid + lo, each chunk
            # bf16-representable; accumulating the three 1.0*chunk products
            # in f32 PSUM reconstructs val bit-exactly.
            u = vals.view(np.uint32)
            hi = (u & np.uint32(0xFFFF0000)).view(f32)
            r1 = vals - hi
            mid = (r1.view(np.uint32) & np.uint32(0xFFFF0000)).view(f32)
            lo = r1 - mid
            vals3 = np.zeros((128, NS * 128), dtype=bf16)
            for i in range(NS):
                kw = K_ws[g][i]
                blk = slice(i * 128, (i + 1) * 128)
                vals3[0:kw, blk] = hi[0:kw, blk].astype(bf16)
                vals3[kw:2 * kw, blk] = mid[0:kw, blk].astype(bf16)
                vals3[2 * kw:3 * kw, blk] = lo[0:kw, blk].astype(bf16)
            m[f"vals{g}"] = vals3
            m[f"ck{g}"] = ck
        m["iota"] = np.broadcast_to(
            np.arange(SUB, dtype=f32), (128, SUB)).copy()
        in_maps.append(m)

    meta = dict(V=V, T=T, B=B, NW=NW, NS=NS, G=G, sub_geom=sub_geom,
                sub_of=sub_of, K_ws=K_ws, KMAX=KMAX, BPC=BPC,
                win_info=win_info)
    return in_maps, meta


def _build_nc(meta):
    from concourse import bacc, mybir

    V, NW, NS, G = meta["V"], meta["NW"], meta["NS"], meta["G"]
    sub_geom, K_ws, KMAX = meta["sub_geom"], meta["K_ws"], meta["KMAX"]
    f32 = mybir.dt.float32

    bf16 = mybir.dt.bfloat16
    nc = bacc.Bacc(None, target_bir_lowering=False, debug=False)
    vocab = [nc.declare_dram_parameter(f"vocab{g}", [128, V], f32, isOutput=False)
             for g in range(G)]
    pgen = [nc.declare_dram_parameter(f"pgen{g}", [128, 1], f32, isOutput=False)
            for g in range(G)]
    vals = [nc.declare_dram_parameter(f"vals{g}", [128, NS * 128], bf16, isOutput=False)
            for g in range(G)]
    ck = [nc.declare_dram_parameter(f"ck{g}", [128, NS], f32, isOutput=False)
          for g in range(G)]
    iota = nc.declare_dram_parameter("iota", [128, SUB], f32, isOutput=False)
    out = [nc.declare_dram_parameter(f"out{g}", [128, V], f32, isOutput=True)
           for g in range(G)]

    from concourse.tile import TileContext

    win_info = meta["win_info"]
    with TileContext(nc) as tc:
        with tc.tile_pool(name="io", bufs=8) as io_pool, \
             tc.tile_pool(name="small", bufs=1) as small, \
             tc.tile_pool(name="vw", bufs=4) as vw_pool, \
             tc.tile_pool(name="oh", bufs=8) as oh_pool, \
             tc.tile_pool(name="psum", bufs=8, space="PSUM") as psum_pool:

            iota_t = small.tile([128, SUB], f32)
            nc.sync.dma_start(out=iota_t[:], in_=iota[:])

            for g in range(G):
                p_t = small.tile([128, 1], f32, tag=f"p{g}")
                nc.sync.dma_start(out=p_t[:], in_=pgen[g][:])
                ck_t = small.tile([128, NS], f32, tag=f"ck{g}")
                nc.sync.dma_start(out=ck_t[:], in_=ck[g][:])

                for w in range(NW):
                    c0w = w * WIN
                    ww = min(WIN, V - c0w)
                    i0, nsub, kw = win_info[g][w]
                    vals_w = vw_pool.tile([128, 8 * 128], bf16, tag="vw")
                    nc.sync.dma_start(
                        out=vals_w[:kw, :nsub * 128],
                        in_=vals[g][:kw, i0 * 128:(i0 + nsub) * 128])
                    t = io_pool.tile([128, WIN], f32, tag="io")
                    nc.sync.dma_start(out=t[:, :ww], in_=vocab[g][:, c0w:c0w + ww])
                    nc.scalar.activation(
                        t[:, :ww], t[:, :ww],
                        mybir.ActivationFunctionType.Copy, scale=p_t[:, :1])
                    for s in range(nsub):
                        i = i0 + s
                        K = K_ws[g][i]
                        if K == 0:
                            continue
                        _, _, c0, wd = sub_geom[i]
                        K3 = 3 * K
                        oh = oh_pool.tile([128, SUB], bf16, tag="oh")
                        nc.vector.tensor_scalar(
                            out=oh[:K3, :wd], in0=iota_t[:K3, :wd],
                            scalar1=ck_t[:K3, i:i + 1], scalar2=None,
                            op0=mybir.AluOpType.is_equal)
                        ps = psum_pool.tile([128, SUB], f32, tag="ps")
                        nc.tensor.matmul(
                            out=ps[:, :wd],
                            lhsT=vals_w[:K3, s * 128:(s + 1) * 128],
                            rhs=oh[:K3, :wd],
                            start=True, stop=True)
                        lo = c0 - c0w
                        nc.vector.tensor_add(
                            out=t[:, lo:lo + wd], in0=t[:, lo:lo + wd],
                            in1=ps[:, :wd])
                    nc.sync.dma_start(out=out[g][:, c0w:c0w + ww], in_=t[:, :ww])
    nc.finalize()
    return nc


def kernel(vocab_ds, attns, p_gens, sources, decoder_batch_len):
    T = int(decoder_batch_len)
    in_maps, meta = _host_prep(vocab_ds, attns, p_gens, sources, T)
    nc = _build_nc(meta)

    from concourse.bass_utils import run_bass_kernel_spmd
    res = run_bass_kernel_spmd(nc, in_maps, list(range(N_CORES)))

    rows, V = np.asarray(vocab_ds).shape
    B, BPC, G = meta["B"], meta["BPC"], meta["G"]
    full = np.empty((rows, V), dtype=np.float32)
    for core in range(N_CORES):
        for g in range(G):
            blk = res.results[core][f"out{g}"]
            for half in range(2):
                b = core * BPC + g * 2 + half
                full[b::B] = blk[half * T:(half + 1) * T]
    return full


# revision 16
# speedup vs baseline: 1.0759x; 1.0759x over previous
"""Trainium2 Bass kernel for pointer-generator final-distribution (scatter_memory).

out[r, v] = p_gens[r] * vocab_ds[r, v]  (+ (1-p_gens[r])*attns[r, l_win]  at
v == sources[l, b(r)], duplicate source ids resolved last-occurrence-wins)

Strategy (8 NeuronCores, SPMD):
  - Shard by batch column: core k owns b in {4k..4k+3}, all T decoder steps
    (rows r = t*B + b). Host pre-gathers rows b-major so device DMAs are
    contiguous; two 128-row groups per core (2 b's x 64 t each).
  - Stream out = p * vocab through SBUF ([128, 4096] tiles, ACT does the
    per-partition scale).
  - The scatter is applied in SBUF before the store via a compact one-hot
    matmul on the (otherwise idle) PE: for each 512-wide subtile, host bakes
    a [K, 128] block of update values (update k x row, block-diagonal over
    the two b's) and relative target columns ck; device builds the one-hot
    [K, 512] with is_equal(iota, ck) and computes proj = vals.T @ onehot in
    PSUM, then DVE adds it into the streamed tile. All products are 1.0*val
    or 0.0*val so the result is exact f32.
"""

import numpy as np

N_CORES = 8
WIN = 4096
SUB = 512


def _host_prep(vocab_ds, attns, p_gens, sources, T):
    f32 = np.float32
    vocab_ds = np.ascontiguousarray(vocab_ds, dtype=f32)
    attns = np.ascontiguousarray(attns, dtype=f32)
    p_gens = np.ascontiguousarray(p_gens, dtype=f32)
    src = np.asarray(sources).astype(np.int64)
    rows, V = vocab_ds.shape
    L, B = src.shape
    assert rows == T * B

    ag = (f32(1.0) - p_gens) * attns  # gated copy dist, [rows, L]

    # winners per batch column: duplicate source ids -> last occurrence wins
    wins = []
    for b in range(B):
        d = {}
        col = src[:, b]
        for l in range(L):
            d[int(col[l])] = l
        cols = np.fromiter(d.keys(), dtype=np.int64)
        ls = np.fromiter(d.values(), dtype=np.int64)
        o = np.argsort(cols)
        wins.append((cols[o], ls[o]))

    NW = (V + WIN - 1) // WIN
    # subtile geometry, shared by all cores/groups
    sub_geom = []  # (w, s, c0_abs, width)
    for w in range(NW):
        ww = min(WIN, V - w * WIN)
        for s in range((ww + SUB - 1) // SUB):
            sub_geom.append((w, s, w * WIN + s * SUB, min(SUB, ww - s * SUB)))
    NS = len(sub_geom)
    sub_of = {}
    for i, (w, s, c0, wd) in enumerate(sub_geom):
        sub_of[(w, s)] = i

    BPC = B // N_CORES  # 4
    G = BPC // 2        # 2 groups of 2 b's

    # bucket updates per (core, g, subtile)
    upd = [[[[] for _ in range(NS)] for _ in range(G)] for _ in range(N_CORES)]
    for core in range(N_CORES):
        for g in range(G):
            for half in range(2):
                b = core * BPC + g * 2 + half
                cols, ls = wins[b]
                for c, l in zip(cols.tolist(), ls.tolist()):
                    w = c // WIN
                    s = (c - w * WIN) // SUB
                    i = sub_of[(w, s)]
                    upd[core][g][i].append((half, c, l))

    # uniform-per-(g, subtile) K across cores
    K_ws = [[max(len(upd[core][g][i]) for core in range(N_CORES)) for i in range(NS)]
            for g in range(G)]
    KMAX = [max(K_ws[g]) if NS else 0 for g in range(G)]
    # per-(g, window): first subtile index, #subtiles, max 3K (partition
    # extent of the JIT vals load for that window; 3x for hi/mid/lo split)
    win_info = []
    for g in range(G):
        wi = []
        for w in range(NW):
            idxs = [i for i, (w2, s2, _, _) in enumerate(sub_geom) if w2 == w]
            i0, nsub = idxs[0], len(idxs)
            kw = max(3 * K_ws[g][i] for i in idxs)
            wi.append((i0, nsub, kw))
        win_info.append(wi)

    # per-core device inputs
    in_maps = []
    for core in range(N_CORES):
        m = {}
        for g in range(G):
            row_idx = []
            for half in range(2):
                b = core * BPC + g * 2 + half
                row_idx.extend(t * B + b for t in range(T))
            row_idx = np.asarray(row_idx)
            m[f"vocab{g}"] = vocab_ds[row_idx]
            m[f"pgen{g}"] = p_gens[row_idx]
            import ml_dtypes
            bf16 = ml_dtypes.bfloat16
            vals = np.zeros((128, NS * 128), dtype=f32)
            ck = np.full((128, NS), -1.0, dtype=f32)
            for i in range(NS):
                w, s, c0, wd = sub_geom[i]
                for k, (half, c, l) in enumerate(upd[core][g][i]):
                    # rows of this b occupy partitions half*T .. half*T+T
                    r0 = half * T
                    vals[k, i * 128 + r0: i * 128 + r0 + T] = ag[row_idx[r0: r0 + T], l]
                    kw = K_ws[g][i]
                    ck[k, i] = f32(c - c0)
                    ck[kw + k, i] = f32(c - c0)
                    ck[2 * kw + k, i] = f32(c - c0)
            # exact 3-way bf16 split: val = hi + mid + lo, each chunk
            # bf16-representable; accumulating the three 1.0*chunk products
            # in f32 PSUM reconstructs val bit-exactly.
            u = vals.view(np.uint32)
            hi = (u & np.uint32(0xFFFF0000)).view(f32)
            r1 = vals - hi
            mid = (r1.view(np.uint32) & np.uint32(0xFFFF0000)).view(f32)
            lo = r1 - mid
            vals3 = np.zeros((128, NS * 128), dtype=bf16)
            for i in range(NS):
                kw = K_ws[g][i]
                blk = slice(i * 128, (i + 1) * 128)
                vals3[0:kw, blk] = hi[0:kw, blk].astype(bf16)
                vals3[kw:2 * kw, blk] = mid[0:kw, blk].astype(bf16)
                vals3[2 * kw:3 * kw, blk] = lo[0:kw, blk].astype(bf16)
            m[f"vals{g}"] = vals3
            m[f"ck{g}"] = ck
        m["iota"] = np.broadcast_to(
            np.arange(SUB, dtype=f32), (128, SUB)).copy()
        in_maps.append(m)

    meta = dict(V=V, T=T, B=B, NW=NW, NS=NS, G=G, sub_geom=sub_geom,
                sub_of=sub_of, K_ws=K_ws, KMAX=KMAX, BPC=BPC,
                win_info=win_info)
    return in_maps, meta


def _build_nc(meta):
    from concourse import bacc, mybir

    V, NW, NS, G = meta["V"], meta["NW"], meta["NS"], meta["G"]
    sub_geom, K_ws, KMAX = meta["sub_geom"], meta["K_ws"], meta["KMAX"]
    f32 = mybir.dt.float32

    bf16 = mybir.dt.bfloat16
    nc = bacc.Bacc(None, target_bir_lowering=False, debug=False)
    vocab = [nc.declare_dram_parameter(f"vocab{g}", [128, V], f32, isOutput=False)
             for g in range(G)]
    pgen = [nc.declare_dram_parameter(f"pgen{g}", [128, 1], f32, isOutput=False)
            for g in range(G)]
    vals = [nc.declare_dram_parameter(f"vals{g}", [128, NS * 128], bf16, isOutput=False)
            for g in range(G)]
    ck = [nc.declare_dram_parameter(f"ck{g}", [128, NS], f32, isOutput=False)
          for g in range(G)]
    iota = nc.declare_dram_parameter("iota", [128, SUB], f32, isOutput=False)
    out = [nc.declare_dram_parameter(f"out{g}", [128, V], f32, isOutput=True)
           for g in range(G)]

    from concourse.tile import TileContext

    win_info = meta["win_info"]
    with TileContext(nc) as tc:
        with tc.tile_pool(name="io", bufs=6) as io_pool, \
             tc.tile_pool(name="small", bufs=1) as small, \
             tc.tile_pool(name="vw", bufs=4) as vw_pool, \
             tc.tile_pool(name="oh", bufs=8) as oh_pool, \
             tc.tile_pool(name="psum", bufs=8, space="PSUM") as psum_pool:

            iota_t = small.tile([128, SUB], f32)
            nc.sync.dma_start(out=iota_t[:], in_=iota[:])

            for g in range(G):
                p_t = small.tile([128, 1], f32, tag=f"p{g}")
                nc.sync.dma_start(out=p_t[:], in_=pgen[g][:])
                ck_t = small.tile([128, NS], f32, tag=f"ck{g}")
                nc.sync.dma_start(out=ck_t[:], in_=ck[g][:])

                for w in range(NW):
                    c0w = w * WIN
                    ww = min(WIN, V - c0w)
                    i0, nsub, kw = win_info[g][w]
                    vals_w = vw_pool.tile([128, 8 * 128], bf16, tag="vw")
                    nc.sync.dma_start(
                        out=vals_w[:kw, :nsub * 128],
                        in_=vals[g][:kw, i0 * 128:(i0 + nsub) * 128])
                    t = io_pool.tile([128, WIN], f32, tag="io")
                    nc.sync.dma_start(out=t[:, :ww], in_=vocab[g][:, c0w:c0w + ww])
                    nc.scalar.activation(
                        t[:, :ww], t[:, :ww],
                        mybir.ActivationFunctionType.Copy, scale=p_t[:, :1])
                    for s in range(nsub):
                        i = i0 + s
                        K = K_ws[g][i]
                        if K == 0:
                            continue
                        _, _, c0, wd = sub_geom[i]
                        K3 = 3 * K
                        oh = oh_pool.tile([128, SUB], bf16, tag="oh")
                        nc.vector.tensor_scalar(
                            out=oh[:K3, :wd], in0=iota_t[:K3, :wd],
                            scalar1=ck_t[:K3, i:i + 1], scalar2=None,
                            op0=mybir.AluOpType.is_equal)
                        ps = psum_pool.tile([128, SUB], f32, tag="ps")
                        nc.tensor.matmul(
                            out=ps[:, :wd],
                            lhsT=vals_w[:K3, s * 128:(s + 1) * 128],
                            rhs=oh[:K3, :wd],
                            start=True, stop=True)
                        lo = c0 - c0w
                        nc.vector.tensor_add(
                            out=t[:, lo:lo + wd], in0=t[:, lo:lo + wd],
                            in1=ps[:, :wd])
                    nc.sync.dma_start(out=out[g][:, c0w:c0w + ww], in_=t[:, :ww])
    nc.finalize()
    return nc


def kernel(vocab_ds, attns, p_gens, sources, decoder_batch_len):
    T = int(decoder_batch_len)
    in_maps, meta = _host_prep(vocab_ds, attns, p_gens, sources, T)
    nc = _build_nc(meta)

    from concourse.bass_utils import run_bass_kernel_spmd
    res = run_bass_kernel_spmd(nc, in_maps, list(range(N_CORES)))

    rows, V = np.asarray(vocab_ds).shape
    B, BPC, G = meta["B"], meta["BPC"], meta["G"]
    full = np.empty((rows, V), dtype=np.float32)
    for core in range(N_CORES):
        for g in range(G):
            blk = res.results[core][f"out{g}"]
            for half in range(2):
                b = core * BPC + g * 2 + half
                full[b::B] = blk[half * T:(half + 1) * T]
    return full
